# revision 1
# baseline (speedup 1.0000x reference)
"""Trainium2 Bass kernel for nn_Aligner: softmax-over-tokens alignment.

    out[b, d] = sum_a sum_t hidden[b, d, t] * softmax_t(-(c[b,t]-ts[b,a])^2/100)[a, t]

Banded algorithm (per batch): exp(-(d/10)^2) vanishes for |d| beyond a band
radius R (adaptively chosen from the data, capped at the f32-underflow bound
102), so for each group of 32 frame-chunks only a narrow window of W token
centers matters (W = 64 for the reference inputs vs T = 512 dense).

Per 128-partition tile (partition = 8-frame chunk, free = [W, 8]):
  DVE   d = cc - tsd            (broadcast views, f32 in / bf16 out)
  ACT   sq = d^2 ; e = exp(-sq/100)           (bf16)
  DVE   s = row-sums over w ; r = 1/s ; maskr = r * group-mask
  PE    uwT[w, group] += e[:, :, i].T @ maskr[:, i]   (normalize fused into
        the frame-reduction matmuls; K=128 mask matmuls sidestep the PE
        base-partition in {0,32,64} constraint)
Epilogue: uwT -> bf16, 16 accumulating matmuls against a window-local
host-gathered transpose of hidden (bf16) give out[d] directly.

Sharding: data-parallel over batch, 2 batches per core on 8 NeuronCores, no
collectives. All device access patterns are value-independent (SPMD-safe);
data dependence lives only in host-prepared gather inputs. Engine choice
(sub on DVE, square on ACT, nothing on GPSIMD) was A/B-tested on hardware —
the GPSIMD/Pool engine is ~an order of magnitude slower than the cost model
claims.

Host prep is index gathers + small shifts only; all O(B*A*W) math runs on
device. Layouts are chosen so every input loads with one DMA of long
contiguous lines per batch slot (DMA issue overhead ~625ns each).
"""
import numpy as np
import ml_dtypes
import concourse.bacc as bacc
import concourse.mybir as mybir
from concourse import tile
from concourse.bass_utils import run_bass_kernel_spmd

B, T, A, D = 16, 512, 4096, 256
TEMP = 10.0
R_BAND_MAX = 102.0      # |d| band: exp(-(102^2)/100) == 0 in f32 (denormal floor)
C = 8                   # frames per partition chunk (sweepable)
G_CANDIDATES = (32, 16, 8)      # partitions per group; escalate down if
                                # clustered centers make windows too wide
N_CORES = 8
BPC = B // N_CORES      # batches per core


dt = mybir.dt

_build_cache = {}

# engine-assignment knobs (A/B-tested on hardware)
SUB_ENGINE = "dve"      # "pool" | "dve" | "split"
SQ_ENGINE = "act"       # "dve" | "act"
BUFS = 3                # main pool depth
G_FORCE = None          # override group size (sweep harness)
TP = 1                  # tiles merged per op chain (1, 2 or 4)
EL = "wi"               # e-tile free layout: "wi" ([W,C]) or "iw" ([C,W])



def _adaptive_band(centers, ts):
    """Tightest band radius R with provably negligible excluded softmax mass.

    Excluded-to-kept mass ratio per row is <= T * exp((maxmin^2 - R^2)/TEMP^2)
    where maxmin = max over frames of distance to the nearest center; choose
    R so that bound is 1e-6, capped at the f32-underflow radius."""
    maxmin = 0.0
    for b in range(B):
        idx = np.searchsorted(centers[b], ts[b])
        lo = centers[b][np.clip(idx - 1, 0, T - 1)]
        hi = centers[b][np.clip(idx, 0, T - 1)]
        dmin = np.minimum(np.abs(ts[b] - lo), np.abs(ts[b] - hi))
        maxmin = max(maxmin, float(dmin.max()))
    r = np.sqrt(maxmin ** 2 + (TEMP ** 2) * np.log(T * 1e6))
    return float(min(r, R_BAND_MAX))


def _plan_windows(centers_b, ts_b, G, r_band):
    """t_starts[n_tiles, n_groups], needed W for one batch. Requires sorted
    centers (reference sorts them); caller falls back to dense if not."""
    n_groups = 128 // G
    n_tiles = A // (128 * C)
    t_starts = np.zeros((n_tiles, n_groups), dtype=np.int64)
    t_ends = np.zeros((n_tiles, n_groups), dtype=np.int64)
    for tau in range(n_tiles):
        for g in range(n_groups):
            f_lo = tau * 128 * C + g * G * C
            f_hi = f_lo + G * C
            seg = ts_b[f_lo:f_hi]
            t_starts[tau, g] = np.searchsorted(centers_b, seg.min() - r_band, "left")
            t_ends[tau, g] = np.searchsorted(centers_b, seg.max() + r_band, "right")
    return t_starts, int((t_ends - t_starts).max())


def _build(W, G, reps=1):
    """Build + compile the SPMD program (identical on all 8 cores).

    reps > 1 repeats the whole computation inside one NEFF — used by the
    test harness to amortize dispatch overhead when timing on hardware.

    Host-side input layouts (chosen so each tensor loads with ONE DMA of
    long contiguous lines per batch slot):
      cc2 [BPC, 128, N_TILES, W]  f32   partition-major center windows
      tsd [BPC, 128, N_TILES, C]  f32   partition-major frame offsets
      hgt [BPC, W, N_GID, D]      bf16  window-local transposed hidden
    """
    key = (W, G, reps, SUB_ENGINE, SQ_ENGINE, BUFS, C, TP, EL)
    if key in _build_cache:
        return _build_cache[key]
    N_GROUPS = 128 // G
    N_TILES = A // (128 * C)
    N_GID = N_TILES * N_GROUPS

    nc = bacc.Bacc("TRN2", target_bir_lowering=False, debug=False,
                   num_devices=N_CORES)
    cc_d = nc.dram_tensor("cc2", [BPC, 128, N_TILES, W], dt.float32,
                          kind="ExternalInput")
    tsd_d = nc.dram_tensor("tsd", [BPC, 128, N_TILES, C], dt.float32,
                           kind="ExternalInput")
    hgt_d = nc.dram_tensor("hgt", [BPC, W, N_GID, D], dt.bfloat16,
                           kind="ExternalInput")
    mask_d = nc.dram_tensor("mask", [128, N_GROUPS], dt.bfloat16,
                            kind="ExternalInput")
    out_d = nc.dram_tensor("out", [BPC, D], dt.float32, kind="ExternalOutput")

    with tile.TileContext(nc) as tc:
        with tc.tile_pool(name="pool", bufs=BUFS) as pool, \
             tc.tile_pool(name="psum", bufs=1, space="PSUM") as psum_pool, \
             tc.tile_pool(name="const", bufs=1) as cpool, \
             nc.allow_low_precision(reason="softmax weights tolerate bf16"):

            # mask[p, g] = 1 if partition p belongs to group g (host input;
            # engine memsets cannot start at partitions that are not 0/32/64)
            mask_t = cpool.tile([128, N_GROUPS], dt.bfloat16)
            nc.sync.dma_start(out=mask_t[:], in_=mask_d.ap())

            import contextlib
            loop_cm = (tc.For_i(0, reps, 1) if reps > 1
                       else contextlib.nullcontext())
            with loop_cm:
              # load both slots' inputs up front (one DMA per tensor per slot)
              cc_alls, tsd_alls, hg_alls = {}, {}, {}
              for slot in range(BPC):
                cc_all = pool.tile([128, N_TILES, W], dt.float32,
                                   tag=f"cc{slot}")
                nc.sync.dma_start(out=cc_all[:], in_=cc_d.ap()[slot])
                tsd_all = pool.tile([128, N_TILES, C], dt.float32,
                                    tag=f"tsd{slot}")
                nc.sync.dma_start(out=tsd_all[:], in_=tsd_d.ap()[slot])
                hg_all = pool.tile([W, N_GID * D], dt.bfloat16,
                                   tag=f"hg{slot}")
                nc.sync.dma_start(
                    out=hg_all[:],
                    in_=hgt_d.ap()[slot].rearrange("w g d -> w (g d)"))

                cc_alls[slot], tsd_alls[slot] = cc_all, tsd_all
                hg_alls[slot] = hg_all

              # slot-major: slot 0's epilogue overlaps slot 1's tiles
              for slot in range(BPC):
                cc_all, tsd_all = cc_alls[slot], tsd_alls[slot]
                hg_all = hg_alls[slot]
                psum_uwT = psum_pool.tile([W, N_GID], dt.float32,
                                          tag=f"uwT{slot}")
                for tau0 in range(0, N_TILES, TP):
                    cc_t = cc_all[:, tau0:tau0 + TP]    # [128, TP, W]
                    tsd_t = tsd_all[:, tau0:tau0 + TP]  # [128, TP, C]

                    if EL == "wi":
                        eshape = [128, TP, W, C]
                        cc_b = cc_t.unsqueeze(3).broadcast_to(eshape)
                        tsd_b = tsd_t.unsqueeze(2).broadcast_to(eshape)
                    else:
                        # w innermost: contiguous DVE row-sum reads and
                        # contiguous PE weight-load slices
                        eshape = [128, TP, C, W]
                        cc_b = cc_t.unsqueeze(2).broadcast_to(eshape)
                        tsd_b = tsd_t.unsqueeze(3).broadcast_to(eshape)

                    if SUB_ENGINE == "pool" or (SUB_ENGINE == "split"
                                                and tau0 % 2 == 0):
                        sub_eng = nc.gpsimd
                    else:
                        sub_eng = nc.vector
                    d_t = pool.tile(eshape, dt.bfloat16, tag="d")
                    sub_eng.tensor_tensor(out=d_t[:], in0=cc_b, in1=tsd_b,
                                          op=mybir.AluOpType.subtract)

                    sq_t = pool.tile(eshape, dt.bfloat16, tag="sq")
                    if SQ_ENGINE == "act":
                        nc.scalar.square(out=sq_t[:], in_=d_t[:])
                    else:
                        nc.vector.tensor_tensor(out=sq_t[:], in0=d_t[:],
                                                in1=d_t[:],
                                                op=mybir.AluOpType.mult)

                    # e = exp(-sq / TEMP^2)
                    e_t = pool.tile(eshape, dt.bfloat16, tag="e")
                    nc.scalar.activation(out=e_t[:], in_=sq_t[:],
                                         func=mybir.ActivationFunctionType.Exp,
                                         scale=-(TEMP ** -2))

                    # softmax row sums over w: s[p, t, i] = sum_w e
                    s_t = pool.tile([128, TP, C], dt.float32, tag="s")
                    sred_in = (e_t[:].transpose([0, 1, 3, 2]) if EL == "wi"
                               else e_t[:])
                    nc.vector.reduce_sum(out=s_t[:], in_=sred_in,
                                         axis=mybir.AxisListType.X)
                    r_t = pool.tile([128, TP, C], dt.bfloat16, tag="r")
                    nc.vector.reciprocal(out=r_t[:], in_=s_t[:])

                    # maskr[p, t, i, g] = r[p, t, i] * mask[p, g]
                    maskr_t = pool.tile([128, TP, C, N_GROUPS], dt.bfloat16,
                                        tag="maskr")
                    nc.vector.tensor_tensor(
                        out=maskr_t[:],
                        in0=r_t[:].unsqueeze(3).broadcast_to(
                            [128, TP, C, N_GROUPS]),
                        in1=mask_t[:].unsqueeze(1).unsqueeze(2).broadcast_to(
                            [128, TP, C, N_GROUPS]),
                        op=mybir.AluOpType.mult)

                    # uwT[w, groups of tau] += sum_i e_i.T @ maskr[:, t, i]
                    for t in range(TP):
                        tau = tau0 + t
                        for i in range(C):
                            if EL == "wi":
                                e_i = e_t[:, t, :, i:i + 1].squeeze(2)
                            else:
                                e_i = e_t[:, t, i]
                            nc.tensor.matmul(
                                out=psum_uwT[:, tau * N_GROUPS:
                                             (tau + 1) * N_GROUPS],
                                lhsT=e_i,
                                rhs=maskr_t[:, t, i],
                                start=(tau == 0 and i == 0),
                                stop=(tau == N_TILES - 1 and i == C - 1))

                uwT_sb = pool.tile([W, N_GID], dt.bfloat16, tag="uwTs")
                nc.scalar.copy(out=uwT_sb[:], in_=psum_uwT[:])

                # out[d] = sum_gid uwT[:, gid].T @ hg_all[:, gid*D:(gid+1)*D]
                psum_out = psum_pool.tile([1, D], dt.float32, tag=f"out{slot}")
                for gid in range(N_GID):
                    nc.tensor.matmul(out=psum_out[:],
                                     lhsT=uwT_sb[:, gid:gid + 1],
                                     rhs=hg_all[:, gid * D:(gid + 1) * D],
                                     start=(gid == 0), stop=(gid == N_GID - 1))

                out_sb = pool.tile([1, D], dt.float32, tag="osb")
                nc.scalar.copy(out=out_sb[:], in_=psum_out[:])
                nc.sync.dma_start(out=out_d.ap()[slot].unsqueeze(0),
                                  in_=out_sb[:])

    nc.compile()
    _build_cache[key] = nc
    return nc


def _prepare(hidden_state, centers, audio_timestamps):
    """Host planning + gathers. Returns (W, in_maps)."""
    hidden_state = np.ascontiguousarray(hidden_state, dtype=np.float32)
    centers = np.ascontiguousarray(centers, dtype=np.float32)
    ts = np.ascontiguousarray(audio_timestamps, dtype=np.float32)

    sorted_ok = all(np.all(np.diff(centers[b]) >= 0) for b in range(B))
    if not sorted_ok:
        raise NotImplementedError("banded aligner kernel assumes sorted centers")

    W = None
    n_tiles = A // (128 * C)
    r_band = _adaptive_band(centers, ts)
    for G in ((G_FORCE,) if G_FORCE else G_CANDIDATES):
        n_groups = 128 // G
        starts = np.zeros((B, n_tiles, n_groups), dtype=np.int64)
        Wg = 0
        for b in range(B):
            st, w = _plan_windows(centers[b], ts[b], G, r_band)
            starts[b] = st
            Wg = max(Wg, w)
        Wg = min((Wg + 3) // 4 * 4, T)
        if Wg <= 128:
            W, all_starts = Wg, starts
            break
    if W is None:
        raise NotImplementedError(
            "center clustering too extreme for the banded kernel")
    N_GROUPS = n_groups
    N_GID = n_tiles * N_GROUPS
    all_starts = np.clip(np.minimum(all_starts, T - W), 0, None)

    mask = np.zeros((128, N_GROUPS), dtype=ml_dtypes.bfloat16)
    for g in range(N_GROUPS):
        mask[g * G:(g + 1) * G, g] = 1.0

    # gathers (vectorized): windows[b, tau, g] -> slice of centers / hidden
    idx = all_starts[..., None] + np.arange(W)          # [B, nt, ng, W]
    cc2 = np.empty((B, 128, n_tiles, W), dtype=np.float32)
    tsd = np.empty((B, 128, n_tiles, C), dtype=np.float32)
    hgt = np.empty((B, W, N_GID, D), dtype=ml_dtypes.bfloat16)
    for b in range(B):
        cwin = centers[b][idx[b]]                       # [nt, ng, W]
        # partition p of tile tau belongs to group p//G
        cw = np.repeat(cwin, G, axis=1)                 # [nt, 128, W]
        tsb = ts[b].reshape(n_tiles, 128, C)
        base = tsb[:, :, 0:1]
        cc2[b] = (cw - base).transpose(1, 0, 2)
        tsd[b] = (tsb - base).transpose(1, 0, 2)
        # hgt[w, gid, d] = hidden[b, d, t_start(gid)+w]
        hg = hidden_state[b][:, idx[b].reshape(N_GID, W)]   # [D, ngid, W]
        hgt[b] = hg.transpose(2, 1, 0).astype(ml_dtypes.bfloat16)

    in_maps = []
    for k in range(N_CORES):
        bs = slice(k * BPC, (k + 1) * BPC)
        in_maps.append({
            "cc2": np.ascontiguousarray(cc2[bs]),
            "tsd": np.ascontiguousarray(tsd[bs]),
            "hgt": np.ascontiguousarray(hgt[bs]),
            "mask": mask,
        })
    return W, G, in_maps


def kernel(hidden_state, centers, audio_timestamps):
    W, G, in_maps = _prepare(hidden_state, centers, audio_timestamps)
    nc = _build(W, G)
    res = run_bass_kernel_spmd(nc, in_maps, core_ids=list(range(N_CORES)))
    out = np.empty((B, D), dtype=np.float32)
    for k in range(N_CORES):
        out[k * BPC:(k + 1) * BPC] = res.results[k]["out"]
    return out



# revision 2
# speedup vs baseline: 1.0123x; 1.0123x over previous
"""Trainium2 Bass kernel v2 for nn_Aligner: softmax-over-tokens alignment.

    out[b, d] = sum_a sum_t hidden[b, d, t] * softmax_t(-(c[b,t]-ts[b,a])^2/100)[a, t]

Since audio_timestamps are arange, frame a = tau*128 + p (p = partition) and
the banded logits factor as a K=6 matmul on the PE:

    -(g_w - o_p)^2/100 = o_p*(g_w/50) + (-g_w^2/100) + (-o_p^2/100)

with g = center - tau*128 (host window gather) and o_p = p.  Each bf16
operand pair is hi/lo split so products carry ~16 mantissa bits.  One ACT
exp pass turns PSUM logits into bf16 weights; DVE row-sums + reciprocal give
per-frame normalizers; per-tau PE matmuls (lhsT = e, rhs = r column) reduce
frames into token-window sums uwT; a final K=128 stacked matmul pass against
host-gathered hidden windows yields out[d].

Per 128-frame block tau the token window (W = 64) is shared across tau pairs
(gid = 256 frames), so uwT columns stack two 64-row gids per PSUM column
(bases 0 / 64) and the epilogue runs 8 K=128 matmuls per batch slot.

Layout/DMA: 3 input DMAs per batch-slot-pair (rhs4x packs lhsT + all logits
rhs rows; hg packs stacked hidden windows), 1 output DMA.  The reps>1 timing
build unrolls the For_i body 2x with alternating input tiles so DMAs of one
unroll overlap compute of the other.

Sharding: data-parallel over batch, 2 batches per core on 8 NeuronCores, no
collectives.  All device access patterns are value-independent (SPMD-safe).
"""
import numpy as np
import ml_dtypes
import concourse.bacc as bacc
import concourse.mybir as mybir
from concourse import tile
from concourse.bass_utils import run_bass_kernel_spmd

B, T, A, D = 16, 512, 4096, 256
TEMP = 10.0
R_BAND_MAX = 102.0
W = 64                  # token window per 256-frame gid
N_TAU = A // 128        # 32 frame blocks per batch
N_GID = A // 256        # 16 tau-pair groups per batch
N_CORES = 8
BPC = B // N_CORES      # batches per core
NCHUNK = 4              # chunks of 8 taus per slot

dt = mybir.dt
bf16 = ml_dtypes.bfloat16

_build_cache = {}


def _adaptive_band(centers, ts):
    """Band radius with provably negligible excluded softmax mass."""
    maxmin = 0.0
    for b in range(B):
        idx = np.searchsorted(centers[b], ts[b])
        lo = centers[b][np.clip(idx - 1, 0, T - 1)]
        hi = centers[b][np.clip(idx, 0, T - 1)]
        dmin = np.minimum(np.abs(ts[b] - lo), np.abs(ts[b] - hi))
        maxmin = max(maxmin, float(dmin.max()))
    r = np.sqrt(maxmin ** 2 + (TEMP ** 2) * np.log(T * 1e6))
    return float(min(r, R_BAND_MAX))


def _split(x):
    """bf16 hi/lo split of float64 array -> (hi, lo) as float32."""
    hi = x.astype(bf16)
    lo = (x - hi.astype(np.float64)).astype(bf16)
    return hi, lo


def _build(W_, G_, reps=1):
    """Build + compile the SPMD program (identical on all 8 cores)."""
    key = (W_, G_, reps)
    if key in _build_cache:
        return _build_cache[key]
    assert W_ == W

    nc = bacc.Bacc("TRN2", target_bir_lowering=False, debug=False,
                   num_devices=N_CORES)
    rhs_d = nc.dram_tensor("rhs4x", [6, 128 + BPC * N_TAU * W], dt.bfloat16,
                           kind="ExternalInput")
    hg_d = nc.dram_tensor("hg", [BPC, 128, (N_GID // 2) * D], dt.bfloat16,
                          kind="ExternalInput")
    out_d = nc.dram_tensor("out", [BPC, D], dt.float32, kind="ExternalOutput")

    UNROLL = 2 if reps > 1 else 1
    assert reps == 1 or reps % UNROLL == 0

    with tile.TileContext(nc) as tc:
        with tc.tile_pool(name="inp", bufs=1) as ipool, \
             tc.tile_pool(name="work", bufs=3) as pool, \
             tc.tile_pool(name="psum", bufs=1, space="PSUM") as psum_pool, \
             nc.allow_low_precision(reason="softmax weights tolerate bf16"):

            import contextlib
            loop_cm = (tc.For_i(0, reps // UNROLL, 1) if reps > 1
                       else contextlib.nullcontext())
            with loop_cm:
              for u in range(UNROLL):
                rhs_t = ipool.tile([6, 128 + BPC * N_TAU * W], dt.bfloat16,
                                   tag=f"rhs{u}")
                nc.sync.dma_start(out=rhs_t[:], in_=rhs_d.ap())
                hg_ts = []
                for s in range(BPC):
                    hg_t = ipool.tile([128, 8 * D], dt.bfloat16,
                                      tag=f"hg{u}{s}")
                    nc.sync.dma_start(out=hg_t[:], in_=hg_d.ap()[s])
                    hg_ts.append(hg_t)

                lhsT = rhs_t[:, 0:128]
                outsb = pool.tile([1, BPC * D], dt.float32, tag="osb")
                pout = psum_pool.tile([1, BPC * D], dt.float32, tag="pout")

                for s in range(BPC):
                    psum_uwT = psum_pool.tile([128, 8], dt.float32,
                                              tag=f"uwT{s}")
                    for c in range(NCHUNK):
                        base = 128 + (s * N_TAU + c * 8) * W
                        psum_l = psum_pool.tile([128, 8, W], dt.float32,
                                                tag=f"l{c % 2}")
                        nc.tensor.matmul(out=psum_l[:],
                                         lhsT=lhsT,
                                         rhs=rhs_t[:, base:base + 8 * W],
                                         start=True, stop=True)
                        e_t = pool.tile([128, 8, W], dt.bfloat16, tag="e")
                        nc.scalar.activation(
                            out=e_t[:], in_=psum_l[:],
                            func=mybir.ActivationFunctionType.Exp)
                        s_t = pool.tile([128, 8], dt.bfloat16, tag="s")
                        nc.vector.reduce_sum(out=s_t[:], in_=e_t[:],
                                             axis=mybir.AxisListType.X)
                        r_t = pool.tile([128, 8], dt.bfloat16, tag="r")
                        nc.vector.reciprocal(out=r_t[:], in_=s_t[:])
                        for t8 in range(8):
                            tau = c * 8 + t8
                            j = tau // 4
                            h = (tau // 2) % 2
                            nc.tensor.matmul(
                                out=psum_uwT[h * 64:(h + 1) * 64, j:j + 1],
                                lhsT=e_t[:, t8],
                                rhs=r_t[:, t8:t8 + 1],
                                start=(tau % 2 == 0), stop=(tau % 2 == 1))

                    uwTs = pool.tile([128, 8], dt.bfloat16, tag=f"uwTs{s}")
                    nc.scalar.copy(out=uwTs[:], in_=psum_uwT[:])
                    for j in range(8):
                        nc.tensor.matmul(
                            out=pout[0:1, s * D:(s + 1) * D],
                            lhsT=uwTs[:, j:j + 1],
                            rhs=hg_ts[s][:, j * D:(j + 1) * D],
                            start=(j == 0), stop=(j == 7))
                    nc.scalar.copy(out=outsb[0:1, s * D:(s + 1) * D],
                                   in_=pout[0:1, s * D:(s + 1) * D])

                nc.sync.dma_start(
                    out=out_d.ap().rearrange("b d -> () (b d)"),
                    in_=outsb[:])

    nc.compile()
    _build_cache[key] = nc
    return nc


def _prepare(hidden_state, centers, audio_timestamps):
    """Host planning + gathers. Returns (W, G, in_maps)."""
    hidden_state = np.ascontiguousarray(hidden_state, dtype=np.float32)
    centers64 = np.ascontiguousarray(centers, dtype=np.float64)
    ts = np.ascontiguousarray(audio_timestamps, dtype=np.float32)

    if not all(np.all(np.diff(centers64[b]) >= 0) for b in range(B)):
        raise NotImplementedError("banded aligner kernel assumes sorted centers")
    if not np.allclose(ts, np.arange(A, dtype=np.float32)[None, :]):
        raise NotImplementedError("kernel assumes arange audio timestamps")

    r_band = _adaptive_band(centers64, ts)

    o = np.arange(128, dtype=np.float64)
    b_hi, b_lo = _split(-o * o / 100.0)

    wins = np.arange(W)
    taus = np.arange(N_TAU)

    rhs_all = np.zeros((B, 6, N_TAU * W), dtype=bf16)
    st_all = np.zeros((B, N_GID), dtype=np.int64)
    for b in range(B):
        c = centers64[b]
        st = np.zeros(N_GID, dtype=np.int64)
        for g in range(N_GID):
            s0 = np.searchsorted(c, g * 256 - r_band, "left")
            en = np.searchsorted(c, g * 256 + 255 + r_band, "right")
            if en - s0 > W:
                raise NotImplementedError("center clustering too extreme")
            st[g] = min(s0, T - W)
        st_all[b] = st
        idx = st[taus // 2][:, None] + wins[None, :]          # [32, 64]
        gw = c[idx] - (taus * 128.0)[:, None]                 # [32, 64]
        h_hi, h_lo = _split(gw / 50.0)
        q_hi, q_lo = _split(-gw * gw / 100.0)
        rhs_all[b, 0] = h_hi.reshape(-1)
        rhs_all[b, 1] = h_lo.reshape(-1)
        rhs_all[b, 2] = q_hi.reshape(-1)
        rhs_all[b, 3] = q_lo.reshape(-1)
        rhs_all[b, 4] = 1.0
        rhs_all[b, 5] = 1.0

    # hg[b, rowp, j*D + d] = hidden[b, d, st[2j + rowp//64] + rowp%64]
    hg_all = np.empty((B, 128, 8 * D), dtype=bf16)
    rowp = np.arange(128)
    for b in range(B):
        idx_all = st_all[b][2 * (np.arange(8))[None, :] + rowp[:, None] // 64] \
            + (rowp % 64)[:, None]                            # [128, 8]
        hgb = hidden_state[b][:, idx_all]                     # [D, 128, 8]
        hg_all[b] = hgb.transpose(1, 2, 0).reshape(128, 8 * D).astype(bf16)

    lhsT_cols = np.zeros((6, 128), dtype=bf16)
    lhsT_cols[0] = o.astype(bf16)
    lhsT_cols[1] = o.astype(bf16)
    lhsT_cols[2] = 1.0
    lhsT_cols[3] = 1.0
    lhsT_cols[4] = b_hi
    lhsT_cols[5] = b_lo

    in_maps = []
    for k in range(N_CORES):
        bs = [2 * k, 2 * k + 1]
        rhs4x = np.empty((6, 128 + BPC * N_TAU * W), dtype=bf16)
        rhs4x[:, 0:128] = lhsT_cols
        for s, b in enumerate(bs):
            lo = 128 + s * N_TAU * W
            rhs4x[:, lo:lo + N_TAU * W] = rhs_all[b]
        hg = np.ascontiguousarray(hg_all[bs])
        in_maps.append({"rhs4x": rhs4x, "hg": hg})
    return W, 256, in_maps


def kernel(hidden_state, centers, audio_timestamps):
    W_, G_, in_maps = _prepare(hidden_state, centers, audio_timestamps)
    nc = _build(W_, G_)
    res = run_bass_kernel_spmd(nc, in_maps, core_ids=list(range(N_CORES)))
    out = np.empty((B, D), dtype=np.float32)
    for k in range(N_CORES):
        out[2 * k:2 * k + 2] = res.results[k]["out"]
    return out
